# revision 1
# baseline (speedup 1.0000x reference)
"""LongcatMoe Trainium2 kernel — expert-parallel sparse MoE across 8 NeuronCores.

Strategy (expert-parallel, per the sharding hint):
  - Host computes the tiny router (fp64 softmax/top-k, ~34 MFLOP) and
    dispatches tokens by top-k expert id: core e receives the tokens routed
    to expert e (padded to capacity C), plus expert e's weights (cast bf16).
  - Each core runs the silu-gated MLP for its expert on its token block:
      y[:, t] = ((silu(Wg.T x_t)) * (Wu.T x_t)).T @ Wd     in [H, C] layout,
    bf16 matmuls with fp32 PSUM accumulation.
  - Host combines: out[tok] += gate_weight * y, plus the zero-expert
    (identity) term zero_w[t] * x[t].

All tensors are laid out host-side so every device DMA is contiguous
per-partition:
  xT  [128, HO, C]      xT[p, ho, t]  = x[idx[t], ho*128+p]
  wg  [IO, 128, HO, 128] wg[j, p, ho, c] = w_gate[ho*128+p, j*128+c]
  wu  same as wg
  wd  [HO, 128, IO, 128] wd[k, p, io, c] = w_down[io*128+p, k*128+c]
  y   [H, C] fp32 output
"""

import os

import numpy as np
import ml_dtypes

T, H, I, E, Z, TOPK = 1024, 2048, 1024, 8, 8, 4
ROUTED_SCALING = 1.0
N_CORES = 8
P = 128
HO = H // P  # 16
IO = I // P  # 8
C = 288      # per-expert token capacity on device (seed-0 max is 278)

_PROGRAM = None
LAST_RESULTS = None  # BassKernelResults of the most recent run (for test harness)


def _build_program():
    import concourse.mybir as mybir
    import concourse.tile as tile
    from concourse import bacc

    f32 = mybir.dt.float32
    bf16 = mybir.dt.bfloat16
    SILU = mybir.ActivationFunctionType.Silu

    nc = bacc.Bacc(
        "TRN2",
        target_bir_lowering=False,
        debug=False,
        enable_asserts=False,
        num_devices=N_CORES,
    )
    xT = nc.dram_tensor("xT", [P, HO, C], bf16, kind="ExternalInput").ap()
    wg = nc.dram_tensor("wg", [IO, P, HO, P], bf16, kind="ExternalInput").ap()
    wu = nc.dram_tensor("wu", [IO, P, HO, P], bf16, kind="ExternalInput").ap()
    wd = nc.dram_tensor("wd", [HO, P, IO, P], bf16, kind="ExternalInput").ap()
    y = nc.dram_tensor("y", [H, C], f32, kind="ExternalOutput").ap()

    with tile.TileContext(nc) as tc:
        with (
            tc.tile_pool(name="px", bufs=1) as px,
            tc.tile_pool(name="pwg", bufs=IO) as pwg,
            tc.tile_pool(name="pwu", bufs=IO) as pwu,
            tc.tile_pool(name="pwd", bufs=HO) as pwd,
            tc.tile_pool(name="pmid", bufs=IO) as pmid,
            tc.tile_pool(name="psg", bufs=2) as psg,
            tc.tile_pool(name="py", bufs=6) as py,
            tc.tile_pool(name="pwrm", bufs=1) as pwrm,
            tc.tile_pool(name="ppg", bufs=2, space="PSUM") as ppg,
            tc.tile_pool(name="ppu", bufs=2, space="PSUM") as ppu,
            tc.tile_pool(name="ppd", bufs=3, space="PSUM") as ppd,
            tc.tile_pool(name="ppw", bufs=1, space="PSUM") as ppw,
        ):
            # PE warmup: keep the tensor engine busy while input DMAs land so
            # the HAM clock-gate reaches 2.4 GHz before the real matmuls.
            wtile = pwrm.tile([P, 512], bf16)
            nc.vector.memset(wtile[:], 0.0)
            pwm = ppw.tile([P, 512], f32)
            for w in range(8):
                nc.tensor.matmul(pwm[:], wtile[:, :P], wtile[:],
                                 start=(w == 0), stop=(w == 7))

            # Input DMAs, emission order = consumption order. Inputs ride the
            # SP HWDGE ring; wd + y outputs ride the ACT ring so the output
            # stream never queues behind input weights.
            xt = px.tile([P, HO, C], bf16)
            wg_t = [pwg.tile([P, HO, P], bf16, name=f"wg{j}", tag="wg")
                    for j in range(IO)]
            wu_t = [pwu.tile([P, HO, P], bf16, name=f"wu{j}", tag="wu")
                    for j in range(IO)]
            wd_t = [pwd.tile([P, IO, P], bf16, name=f"wd{k}", tag="wd")
                    for k in range(HO)]

            nc.sync.dma_start(wg_t[0][:], wg[0])
            XC = HO // 4  # xt loaded in 4 chunks of 4 h-slices
            nc.sync.dma_start(xt[:, 0:XC, :], xT[:, 0:XC, :])
            nc.sync.dma_start(wu_t[0][:], wu[0])
            for c in range(1, 4):
                nc.sync.dma_start(xt[:, c * XC:(c + 1) * XC, :],
                                  xT[:, c * XC:(c + 1) * XC, :])
            for j in range(1, IO):
                nc.sync.dma_start(wg_t[j][:], wg[j])
                nc.sync.dma_start(wu_t[j][:], wu[j])
            for k in range(HO):
                nc.scalar.dma_start(wd_t[k][:], wd[k])

            # Phase 1: mid[j] = silu(x @ Wg_j) * (x @ Wu_j) in [I, C] layout.
            mids = []
            for j in range(IO):
                pg = ppg.tile([P, C], f32)
                pu = ppu.tile([P, C], f32)
                for h in range(HO):
                    nc.tensor.matmul(
                        pg[:], wg_t[j][:, h, :], xt[:, h, :],
                        start=(h == 0), stop=(h == HO - 1),
                    )
                for h in range(HO):
                    nc.tensor.matmul(
                        pu[:], wu_t[j][:, h, :], xt[:, h, :],
                        start=(h == 0), stop=(h == HO - 1),
                    )
                sg = psg.tile([P, C], f32)
                nc.scalar.activation(sg[:], pg[:], SILU)
                mid = pmid.tile([P, C], bf16)
                nc.vector.tensor_mul(out=mid[:], in0=sg[:], in1=pu[:])
                mids.append(mid)

            # Phase 2: y[k] = sum_j Wd[j, k].T @ mid[j] in [H, C] layout.
            for k in range(HO):
                pd = ppd.tile([P, C], f32)
                for j in range(IO):
                    nc.tensor.matmul(
                        pd[:], wd_t[k][:, j, :], mids[j][:],
                        start=(j == 0), stop=(j == IO - 1),
                    )
                ty = py.tile([P, C], f32)
                nc.vector.tensor_copy(out=ty[:], in_=pd[:])
                nc.scalar.dma_start(y[k * P:(k + 1) * P, :], ty[:])

    nc.compile()
    return nc


def _route(x, router_w, corr_bias):
    """fp64 router: returns (topk_idx [T,K], topk_w [T,K])."""
    xl = x.astype(np.float64)
    logits = xl @ router_w.astype(np.float64).T
    logits -= logits.max(axis=1, keepdims=True)
    p = np.exp(logits)
    p /= p.sum(axis=1, keepdims=True)
    sel = p + corr_bias.astype(np.float64)
    topk_idx = np.argsort(-sel, axis=1, kind="stable")[:, :TOPK]
    topk_w = np.take_along_axis(p, topk_idx, axis=1) * ROUTED_SCALING
    return topk_idx, topk_w


def kernel(hidden_states, router_w, corr_bias, w_gate, w_up, w_down):
    global _PROGRAM, LAST_RESULTS
    x = np.asarray(hidden_states, dtype=np.float32)
    router_w = np.asarray(router_w, dtype=np.float32)
    corr_bias = np.asarray(corr_bias, dtype=np.float32)
    w_gate = np.asarray(w_gate, dtype=np.float32)
    w_up = np.asarray(w_up, dtype=np.float32)
    w_down = np.asarray(w_down, dtype=np.float32)

    topk_idx, topk_w = _route(x, router_w, corr_bias)
    routed = topk_idx < E
    zero_w = (topk_w * (~routed)).sum(axis=1)  # [T] fp64

    bf = ml_dtypes.bfloat16
    x16 = x.astype(bf)

    # Dispatch: token list + gate weight per expert; overflow beyond C
    # falls back to an exact host computation (empty for the spec'd data).
    idx_list, w_list, overflow = [], [], []
    for e in range(E):
        toks, kpos = np.nonzero(topk_idx == e)
        we = topk_w[toks, kpos]
        if len(toks) > C:
            overflow.append((e, toks[C:], we[C:]))
            toks, we = toks[:C], we[:C]
        idx_list.append(toks)
        w_list.append(we)

    in_maps = []
    for e in range(E):
        toks = idx_list[e]
        n = len(toks)
        xg = np.zeros((C, H), dtype=bf)
        xg[:n] = x16[toks]
        xTd = np.ascontiguousarray(
            xg.T.reshape(HO, P, C).transpose(1, 0, 2))
        wgd = np.ascontiguousarray(
            w_gate[e].astype(bf).reshape(HO, P, IO, P).transpose(2, 1, 0, 3))
        wud = np.ascontiguousarray(
            w_up[e].astype(bf).reshape(HO, P, IO, P).transpose(2, 1, 0, 3))
        wdd = np.ascontiguousarray(
            w_down[e].astype(bf).reshape(IO, P, HO, P).transpose(2, 1, 0, 3))
        in_maps.append({"xT": xTd, "wg": wgd, "wu": wud, "wd": wdd})

    if _PROGRAM is None:
        _PROGRAM = _build_program()

    from concourse.bass_utils import run_bass_kernel_spmd

    kw = {}
    if os.environ.get("MOE_KERNEL_TRACE", "") == "1":
        kw = dict(trace=True, trace_cores=list(range(N_CORES)))
    res = run_bass_kernel_spmd(
        _PROGRAM, in_maps, core_ids=list(range(N_CORES)), **kw)
    LAST_RESULTS = res

    out = np.zeros((T, H), dtype=np.float64)
    for e in range(E):
        n = len(idx_list[e])
        if n:
            ye = res.results[e]["y"]  # [H, C] fp32
            out[idx_list[e]] += w_list[e][:, None] * ye[:, :n].T.astype(np.float64)
    for e, toks, ws in overflow:
        xt = x[toks]
        g = xt @ w_gate[e]
        u = xt @ w_up[e]
        mid = (g / (1.0 + np.exp(-g))) * u
        out[toks] += ws[:, None] * (mid @ w_down[e]).astype(np.float64)
    out += zero_w[:, None] * x.astype(np.float64)
    return out.astype(np.float32)

